# revision 1
# baseline (speedup 1.0000x reference)
"""Multi-head attention kernel for Trainium2, SPMD over 8 NeuronCores.

Sharding: data-parallel over batch (2 groups of 4 cores) x sequence-parallel
over the key/value length within each group (4 slices of 2048). Each core
computes, for its (batch, k-slice): Q/K/V projections (all heads), masked
softmax numerators/denominators over its k-slice, the attention-weighted
values, and a partial final projection. Denominators are AllReduce'd within
each 4-core group on device (split in two so the first overlaps attention);
the 4 partial projected outputs per batch are summed on the host.

Layout notes: activations/weights are cast to bf16 during the DMA load and
transposed on the TensorE (contraction dims must sit on partitions); scores
are computed transposed ([k, q]) so the exp output is directly consumable as
the stationary operand of the AV matmul; the softmax denominator comes from
a ones-column appended to V; no max-subtraction is needed (scores are O(1)),
and masking is a multiplicative bf16 mask applied after exp (exactly
equivalent to the -1e30 additive mask).
"""

import sys

if "/opt/trn_rl_repo" not in sys.path:
    sys.path.insert(0, "/opt/trn_rl_repo")

from contextlib import ExitStack

import numpy as np

import concourse.bass as bass
import concourse.mybir as mybir
import concourse.tile as tile
from concourse import bacc
from concourse.masks import make_identity

B, QL, KL, D, H = 2, 512, 8192, 1024, 8
HD = D // H  # 128
NCORES = 8
GROUPS = [[0, 1, 2, 3], [4, 5, 6, 7]]
KSH = KL // 4  # 2048 k rows per core
SCALE = 1.0 / float(np.sqrt(HD))

F32 = mybir.dt.float32
BF16 = mybir.dt.bfloat16
U8 = mybir.dt.uint8
P = 128
KC = KSH // P  # 16 k chunks of 128
QB = QL // P  # 4 q blocks


def ensure_ntff_hook():
    """Provide antenv.axon_hooks (missing in this image) so trace=True works.

    Mirrors trn_agent_boot._ntff_profile_via_ctypes against the local
    libaxon_pjrt.so. No-op if the real module exists or the .so is absent.
    """
    try:
        import antenv.axon_hooks  # noqa: F401

        return
    except ImportError:
        pass
    import contextlib
    import ctypes
    import types

    mod = types.ModuleType("antenv.axon_hooks")
    holder = [None]
    mod.set_axon_ntff_profile_hook = lambda h: holder.__setitem__(0, h)
    mod.get_axon_ntff_profile_hook = lambda: holder[0]
    try:
        lib = ctypes.CDLL("/opt/axon/libaxon_pjrt.so")
        if hasattr(lib, "axon_start_nrt_profile"):
            lib.axon_start_nrt_profile.argtypes = [
                ctypes.POINTER(ctypes.c_int64),
                ctypes.c_size_t,
            ]
            lib.axon_start_nrt_profile.restype = ctypes.c_int64
            lib.axon_stop_nrt_profile.argtypes = [ctypes.c_char_p]
            lib.axon_stop_nrt_profile.restype = ctypes.c_int64

            @contextlib.contextmanager
            def _hook(output_dir, device_ids):
                import jax

                jax.devices()
                if device_ids:
                    ids = (ctypes.c_int64 * len(device_ids))(*device_ids)
                    rc = lib.axon_start_nrt_profile(ids, len(device_ids))
                else:
                    rc = lib.axon_start_nrt_profile(None, 0)
                if rc != 0:
                    raise RuntimeError(f"axon_start_nrt_profile rc={rc}")
                try:
                    yield
                finally:
                    n = lib.axon_stop_nrt_profile(str(output_dir).encode())
                    print(f"ntff profile: {n} file(s) -> {output_dir}")

            holder[0] = _hook
    except OSError:
        pass
    sys.modules["antenv.axon_hooks"] = mod
    try:
        import antenv

        antenv.axon_hooks = mod
    except ImportError:
        pass


def build_attention_kernel():
    nc = bacc.Bacc(
        "TRN2", target_bir_lowering=False, debug=False, num_devices=NCORES
    )

    xq = nc.declare_dram_parameter("xq", [QL, D], F32, isOutput=False)
    xk = nc.declare_dram_parameter("xk", [KSH, D], F32, isOutput=False)
    xv = nc.declare_dram_parameter("xv", [KSH, D], F32, isOutput=False)
    msk = nc.declare_dram_parameter("msk", [QL, KSH], U8, isOutput=False)
    wq = nc.declare_dram_parameter("wq", [D, D], F32, isOutput=False)
    wk = nc.declare_dram_parameter("wk", [D, D], F32, isOutput=False)
    wv = nc.declare_dram_parameter("wv", [D, D], F32, isOutput=False)
    wf = nc.declare_dram_parameter("wf", [D, D], F32, isOutput=False)
    out = nc.declare_dram_parameter("out", [QL, D], F32, isOutput=True)

    with tile.TileContext(nc) as tc, ExitStack() as ctx:
        consts = ctx.enter_context(tc.tile_pool(name="consts", bufs=1))
        ident = consts.tile([P, P], BF16)
        make_identity(nc, ident)

        # Persistent operand tiles (single-buffered, live for the kernel).
        persist = ctx.enter_context(tc.tile_pool(name="persist", bufs=1))
        wfT = persist.tile([P, H, D], BF16)  # [din in h-chunk, h, dout]
        kT = persist.tile([P, H, KSH], BF16)  # [hd, head, krow]
        qT = persist.tile([P, H, QL], BF16)  # [hd, head, q]
        v_sb = persist.tile([P, KC, H, HD + 1], BF16)  # [krow, kc, h, hd+1]
        maskT = persist.tile([P, KC, QL], BF16)  # [k, kc, q]
        num_sb = persist.tile([P, H, QB, HD], BF16)  # [q, head, qb, hd]
        den0 = persist.tile([P, 12], F32)  # heads 0-2, [q, h*4+qb]
        den1 = persist.tile([P, 20], F32)  # heads 3-7, [q, (h-3)*4+qb]
        rden0 = persist.tile([P, 12], F32)
        rden1 = persist.tile([P, 20], F32)
        sumT = persist.tile([P, H, QL], BF16)  # [hd, head, q]

        wts = ctx.enter_context(tc.tile_pool(name="wts", bufs=1))
        loads = ctx.enter_context(tc.tile_pool(name="loads", bufs=3))
        xts = ctx.enter_context(tc.tile_pool(name="xts", bufs=2))
        mn_pool = ctx.enter_context(tc.tile_pool(name="mn_pool", bufs=4))
        probs_pool = ctx.enter_context(tc.tile_pool(name="probs", bufs=3))
        small = ctx.enter_context(tc.tile_pool(name="small", bufs=4))
        outp = ctx.enter_context(tc.tile_pool(name="outp", bufs=3))
        dram = ctx.enter_context(tc.tile_pool(name="dram", bufs=1, space="DRAM"))

        # One PSUM pool, 8 banks: mm 2x2 + av 4x1. Everything except the AV
        # accumulators shares the [128, 2, 512] "mm" slots.
        psum = ctx.enter_context(tc.tile_pool(name="psum", bufs=1, space="PSUM"))

        def mm_tile(name, dtype=F32):
            return psum.tile([P, 2, 512], dtype, tag="mm", bufs=2, name=name)

        def transpose_w(w_dram, dst, wname):
            """dst[p, cc, dout] = w[dout, cc*128+p] (i.e. dst = W^T), bf16."""
            wns = []
            for rg in range(2):
                wn = loads.tile([P, 4, D], BF16, tag="ld", name=f"wn_{wname}{rg}")
                nc.gpsimd.dma_start(
                    out=wn,
                    in_=w_dram[rg * 512 : (rg + 1) * 512, :].rearrange(
                        "(a p) d -> p a d", p=P
                    ),
                )
                wns.append(wn)
            for cc in range(H):
                pst = mm_tile(f"wt_{wname}_{cc}", BF16)
                for rg in range(2):
                    for j in range(4):
                        nc.tensor.transpose(
                            pst[:, rg, j * P : (j + 1) * P],
                            wns[rg][:, j, cc * P : (cc + 1) * P],
                            ident,
                        )
                nc.vector.tensor_copy(
                    dst[:, cc, :], pst[:].rearrange("p a b -> p (a b)")
                )

        # --- Q path: xq load first (smallest), then Wq ---
        xqn = loads.tile([P, 4, D], BF16, tag="ld")
        nc.gpsimd.dma_start(out=xqn, in_=xq.rearrange("(a p) d -> p a d", p=P))
        wqT = wts.tile([P, H, D], BF16, tag="wT", name="wqT")
        transpose_w(wq, wqT, "q")

        xqT = xts.tile([P, H, QL], BF16, tag="xT")
        for cc2 in range(H // 2):
            pst = mm_tile(f"xqt_{cc2}", BF16)
            for half in range(2):
                cc = cc2 * 2 + half
                for j in range(4):
                    nc.tensor.transpose(
                        pst[:, half, j * P : (j + 1) * P],
                        xqn[:, j, cc * P : (cc + 1) * P],
                        ident,
                    )
            nc.vector.tensor_copy(xqT[:, cc2 * 2 : cc2 * 2 + 2, :], pst[:])
        for m2 in range(H // 2):
            pq = mm_tile(f"pq_{m2}")
            for half in range(2):
                m = m2 * 2 + half
                for cc in range(H):
                    nc.tensor.matmul(
                        pq[:, half, :],
                        wqT[:, cc, m * P : (m + 1) * P],
                        xqT[:, cc, :],
                        start=(cc == 0),
                        stop=(cc == H - 1),
                    )
            nc.any.tensor_copy(out=qT[:, m2 * 2 : m2 * 2 + 2, :], in_=pq[:])

        # --- mask: load+cast per q-block, transpose to [k, q] on PE ---
        mn_tiles = []
        for qb in range(QB):
            mn = mn_pool.tile([P, KSH], BF16, tag="mn", name=f"mn_{qb}")
            nc.gpsimd.dma_start(out=mn, in_=msk[qb * P : (qb + 1) * P, :])
            mn_tiles.append(mn)
        for kc2 in range(KC // 2):
            pst = mm_tile(f"mt_{kc2}", BF16)
            for half in range(2):
                kc = kc2 * 2 + half
                for qb in range(QB):
                    nc.tensor.transpose(
                        pst[:, half, qb * P : (qb + 1) * P],
                        mn_tiles[qb][:, kc * P : (kc + 1) * P],
                        ident,
                    )
            nc.any.tensor_copy(out=maskT[:, kc2 * 2 : kc2 * 2 + 2, :], in_=pst[:])

        # --- Wk, then the K path (stream xk in 512-row chunks) ---
        wkT = wts.tile([P, H, D], BF16, tag="wT", name="wkT")
        transpose_w(wk, wkT, "k")

        for c4 in range(KSH // 512):
            xkn = loads.tile([P, 4, D], BF16, tag="ld", name=f"xkn_{c4}")
            nc.gpsimd.dma_start(
                out=xkn,
                in_=xk[c4 * 512 : (c4 + 1) * 512, :].rearrange("(a p) d -> p a d", p=P),
            )
            xkT = xts.tile([P, H, 512], BF16, tag="xT", name=f"xkT_{c4}")
            for cc2 in range(H // 2):
                pst = mm_tile(f"xkt_{c4}_{cc2}", BF16)
                for half in range(2):
                    cc = cc2 * 2 + half
                    for j in range(4):
                        nc.tensor.transpose(
                            pst[:, half, j * P : (j + 1) * P],
                            xkn[:, j, cc * P : (cc + 1) * P],
                            ident,
                        )
                nc.vector.tensor_copy(xkT[:, cc2 * 2 : cc2 * 2 + 2, :], pst[:])
            for m2 in range(H // 2):
                pk = mm_tile(f"pk_{c4}_{m2}")
                for half in range(2):
                    m = m2 * 2 + half
                    for cc in range(H):
                        nc.tensor.matmul(
                            pk[:, half, :],
                            wkT[:, cc, m * P : (m + 1) * P],
                            xkT[:, cc, :],
                            start=(cc == 0),
                            stop=(cc == H - 1),
                        )
                nc.any.tensor_copy(
                    out=kT[:, m2 * 2 : m2 * 2 + 2, c4 * 512 : (c4 + 1) * 512],
                    in_=pk[:],
                )

        # --- Wv, then the V path ---
        wvT = wts.tile([P, H, D], BF16, tag="wT", name="wvT")
        transpose_w(wv, wvT, "v")

        for c4 in range(KSH // 512):
            xvn = loads.tile([P, 4, D], BF16, tag="ld", name=f"xvn_{c4}")
            nc.gpsimd.dma_start(
                out=xvn,
                in_=xv[c4 * 512 : (c4 + 1) * 512, :].rearrange("(a p) d -> p a d", p=P),
            )
            xvT = xts.tile([P, H, 512], BF16, tag="xT", name=f"xvT_{c4}")
            for cc2 in range(H // 2):
                pst = mm_tile(f"xvt_{c4}_{cc2}", BF16)
                for half in range(2):
                    cc = cc2 * 2 + half
                    for j in range(4):
                        nc.tensor.transpose(
                            pst[:, half, j * P : (j + 1) * P],
                            xvn[:, j, cc * P : (cc + 1) * P],
                            ident,
                        )
                nc.vector.tensor_copy(xvT[:, cc2 * 2 : cc2 * 2 + 2, :], pst[:])
            for mkl in range(4):
                mk = c4 * 4 + mkl
                pv = mm_tile(f"pv_{mk}")
                for n in range(2):
                    for cc in range(H):
                        nc.tensor.matmul(
                            pv[:, n, :],
                            xvT[:, cc, mkl * P : (mkl + 1) * P],
                            wvT[:, cc, n * 512 : (n + 1) * 512],
                            start=(cc == 0),
                            stop=(cc == H - 1),
                        )
                nc.any.tensor_copy(
                    out=v_sb[:, mk, :, 0:HD],
                    in_=pv[:].rearrange("p a (b c) -> p (a b) c", b=4),
                )
        nc.vector.memset(v_sb[:, :, :, HD], 1.0)

        transpose_w(wf, wfT, "f")

        # --- attention per head; exp batched over 2 k-chunks ---
        def attention_head(h, den_tile):
            avs = [
                psum.tile([P, HD + 1], F32, tag="av", bufs=4, name=f"av_{h}_{qb}")
                for qb in range(QB)
            ]
            for kc2 in range(KC // 2):
                ps = mm_tile(f"ps_{h}_{kc2}")
                for half in range(2):
                    kc = kc2 * 2 + half
                    nc.tensor.matmul(
                        ps[:, half, :],
                        kT[:, h, kc * P : (kc + 1) * P],
                        qT[:, h, :],
                        start=True,
                        stop=True,
                    )
                probs = probs_pool.tile(
                    [P, 2, 512], BF16, tag="probs", name=f"pr_{h}_{kc2}"
                )
                nc.scalar.activation(
                    probs[:], ps[:], mybir.ActivationFunctionType.Exp, scale=SCALE
                )
                nc.vector.tensor_mul(
                    probs[:], probs[:], maskT[:, kc2 * 2 : kc2 * 2 + 2, :]
                )
                for half in range(2):
                    kc = kc2 * 2 + half
                    for qb in range(QB):
                        nc.tensor.matmul(
                            avs[qb][:],
                            probs[:, half, qb * P : (qb + 1) * P],
                            v_sb[:, kc, h, :],
                            start=(kc == 0),
                            stop=(kc == KC - 1),
                        )
            hh = h if h < 3 else h - 3
            for qb in range(QB):
                nc.any.tensor_copy(out=num_sb[:, h, qb, :], in_=avs[qb][:, 0:HD])
                nc.any.tensor_copy(
                    out=den_tile[:, hh * 4 + qb : hh * 4 + qb + 1],
                    in_=avs[qb][:, HD : HD + 1],
                )

        def den_allreduce(den_tile, rden_tile, idx):
            ncols = den_tile.shape[-1]
            den_in = dram.tile([P, ncols], F32, name=f"den_in{idx}")
            den_out = dram.tile([P, ncols], F32, name=f"den_out{idx}")
            nc.sync.dma_start(out=den_in[:], in_=den_tile[:])
            nc.gpsimd.collective_compute(
                "AllReduce",
                mybir.AluOpType.add,
                replica_groups=GROUPS,
                ins=[den_in.opt()],
                outs=[den_out.opt()],
            )
            nc.sync.dma_start(out=rden_tile[:], in_=den_out[:])
            # guard fully-masked rows (reference wipes them to 0): 0/eps -> 0
            nc.vector.tensor_scalar_max(rden_tile[:], rden_tile[:], 1e-30)
            nc.vector.reciprocal(rden_tile[:], rden_tile[:])

        def norm_head(h, rden_tile):
            hh = h if h < 3 else h - 3
            snorms = []
            for qb in range(QB):
                snorm = small.tile([P, HD], BF16, tag="snorm", name=f"sn_{h}_{qb}")
                nc.vector.tensor_scalar_mul(
                    snorm[:],
                    num_sb[:, h, qb, :],
                    rden_tile[:, hh * 4 + qb : hh * 4 + qb + 1],
                )
                snorms.append(snorm)
            pst = mm_tile(f"st_{h}", BF16)
            for qb in range(QB):
                nc.tensor.transpose(
                    pst[:, 0, qb * P : (qb + 1) * P], snorms[qb][:], ident
                )
            nc.any.tensor_copy(out=sumT[:, h, :], in_=pst[:, 0, :])

        for h in range(3):
            attention_head(h, den0)
        den_allreduce(den0, rden0, 0)
        for h in range(3, H):
            attention_head(h, den1)
        for h in range(3):
            norm_head(h, rden0)
        den_allreduce(den1, rden1, 1)
        for h in range(3, H):
            norm_head(h, rden1)

        # First 4 output tiles accumulate heads 0-2 on the freed AV psum
        # slots right after norm of those heads, overlapping the second
        # collective; heads 3-7 complete the groups afterwards.
        po_av = {}
        for qb in range(2):
            for n in range(2):
                po = psum.tile([P, 512], F32, tag="av", bufs=4, name=f"poa_{qb}_{n}")
                po_av[(qb, n)] = po
                for h in range(3):
                    nc.tensor.matmul(
                        po[:],
                        sumT[:, h, qb * P : (qb + 1) * P],
                        wfT[:, h, n * 512 : (n + 1) * 512],
                        start=(h == 0),
                        stop=False,
                    )

        def out_dma(eng, qb, n, ot):
            eng.dma_start(
                out=out[qb * P : (qb + 1) * P, n * 512 : (n + 1) * 512],
                in_=ot[:],
            )

        engs = [nc.sync, nc.scalar]
        for i, ((qb, n), po) in enumerate(po_av.items()):
            for h in range(3, H):
                nc.tensor.matmul(
                    po[:],
                    sumT[:, h, qb * P : (qb + 1) * P],
                    wfT[:, h, n * 512 : (n + 1) * 512],
                    start=False,
                    stop=(h == H - 1),
                )
            ot = outp.tile([P, 512], F32, tag="out", name=f"ota_{qb}_{n}")
            nc.any.tensor_copy(out=ot[:], in_=po[:])
            out_dma(engs[i % 2], qb, n, ot)
        for n in range(2):
            po = mm_tile(f"po_b_{n}")
            for half in range(2):
                qb = 2 + half
                for h in range(H):
                    nc.tensor.matmul(
                        po[:, half, :],
                        sumT[:, h, qb * P : (qb + 1) * P],
                        wfT[:, h, n * 512 : (n + 1) * 512],
                        start=(h == 0),
                        stop=(h == H - 1),
                    )
            for half in range(2):
                qb = 2 + half
                ot = outp.tile([P, 512], F32, tag="out", name=f"otb_{qb}_{n}")
                nc.any.tensor_copy(out=ot[:], in_=po[:, half, :])
                out_dma(engs[(qb + n) % 2], qb, n, ot)

    nc.compile()
    return nc


_NC_CACHE = None


def _get_nc():
    global _NC_CACHE
    if _NC_CACHE is None:
        _NC_CACHE = build_attention_kernel()
    return _NC_CACHE


def make_in_maps(inputs):
    inputs = {k: np.asarray(v) for k, v in inputs.items()}
    in_maps = []
    for c in range(NCORES):
        b, s = c // 4, c % 4
        in_maps.append(
            {
                "xq": np.ascontiguousarray(inputs["inputs_q"][b]),
                "xk": np.ascontiguousarray(
                    inputs["inputs_k"][b, s * KSH : (s + 1) * KSH]
                ),
                "xv": np.ascontiguousarray(
                    inputs["inputs_v"][b, s * KSH : (s + 1) * KSH]
                ),
                "msk": np.ascontiguousarray(
                    inputs["attention_mask"][b, :, s * KSH : (s + 1) * KSH]
                ).view(np.uint8),
                "wq": np.ascontiguousarray(inputs["Wq"]),
                "wk": np.ascontiguousarray(inputs["Wk"]),
                "wv": np.ascontiguousarray(inputs["Wv"]),
                "wf": np.ascontiguousarray(inputs["Wf"]),
            }
        )
    return in_maps


def gather_out(results):
    out = np.zeros((B, QL, D), np.float32)
    for c in range(NCORES):
        out[c // 4] += results[c]["out"]
    return out


def kernel(**inputs) -> np.ndarray:
    ensure_ntff_hook()  # defensive: BASS_TRACE=1 in env would need the shim
    from concourse.bass_utils import run_bass_kernel_spmd

    nc = _get_nc()
    in_maps = make_in_maps(inputs)
    res = run_bass_kernel_spmd(nc, in_maps, list(range(NCORES)))
    return gather_out(res.results)



# revision 2
# speedup vs baseline: 1.2055x; 1.2055x over previous
"""Multi-head attention kernel for Trainium2, SPMD over 8 NeuronCores.

Sharding: data-parallel over batch (2 groups of 4 cores) x sequence-parallel
over the key/value length within each group (4 slices of 2048). Each core
computes K/V projections + masked-softmax attention for all heads on its
k-slice. The Q projection and the final projection are additionally sharded
by heads within each group (2 heads per core): Q head-slices are AllGather'd
(hidden under the K projection), and the per-head attention numerators (with
the softmax denominator riding along as a 129th column) are ReduceScatter'd
in two rounds so each core normalizes and projects only its own 2 heads.
The 4 partial outputs per batch group are summed on the host.

Layout notes: all activations/weights/mask are pre-transposed and pre-cast
to bf16 on the host, so the device does zero layout transposes (contraction
dims arrive on partitions); scores are computed transposed ([k, q]) so the
exp output is directly the stationary operand of the AV matmul; masking is
a multiplicative bf16 mask applied after exp (equivalent to the -1e30
additive mask); no max-subtraction is needed (scores are O(1)).
"""

import sys

if "/opt/trn_rl_repo" not in sys.path:
    sys.path.insert(0, "/opt/trn_rl_repo")

from contextlib import ExitStack

import ml_dtypes
import numpy as np

import concourse.bass as bass  # noqa: F401
import concourse.mybir as mybir
import concourse.tile as tile
from concourse import bacc
from concourse.masks import make_identity

B, QL, KL, D, H = 2, 512, 8192, 1024, 8
HD = D // H  # 128
NCORES = 8
GROUPS = [[0, 1, 2, 3], [4, 5, 6, 7]]
KSH = KL // 4  # 2048 k rows per core
SCALE = 1.0 / float(np.sqrt(HD))

F32 = mybir.dt.float32
BF16 = mybir.dt.bfloat16
P = 128
KC = KSH // P  # 16 k chunks of 128
QB = QL // P  # 4 q blocks
DB = D // P  # 8 d-in blocks


def qslot(h):
    """qT column-slot of head h after the AllGather (core g owns heads g, g+4)."""
    return 2 * (h % 4) + h // 4


def ensure_ntff_hook():
    """Provide antenv.axon_hooks (missing in this image) so trace=True works.

    Mirrors trn_agent_boot._ntff_profile_via_ctypes against the local
    libaxon_pjrt.so. No-op if the real module exists or the .so is absent.
    """
    try:
        import antenv.axon_hooks  # noqa: F401

        return
    except ImportError:
        pass
    import contextlib
    import ctypes
    import types

    mod = types.ModuleType("antenv.axon_hooks")
    holder = [None]
    mod.set_axon_ntff_profile_hook = lambda h: holder.__setitem__(0, h)
    mod.get_axon_ntff_profile_hook = lambda: holder[0]
    try:
        lib = ctypes.CDLL("/opt/axon/libaxon_pjrt.so")
        if hasattr(lib, "axon_start_nrt_profile"):
            lib.axon_start_nrt_profile.argtypes = [
                ctypes.POINTER(ctypes.c_int64),
                ctypes.c_size_t,
            ]
            lib.axon_start_nrt_profile.restype = ctypes.c_int64
            lib.axon_stop_nrt_profile.argtypes = [ctypes.c_char_p]
            lib.axon_stop_nrt_profile.restype = ctypes.c_int64

            @contextlib.contextmanager
            def _hook(output_dir, device_ids):
                import jax

                jax.devices()
                if device_ids:
                    ids = (ctypes.c_int64 * len(device_ids))(*device_ids)
                    rc = lib.axon_start_nrt_profile(ids, len(device_ids))
                else:
                    rc = lib.axon_start_nrt_profile(None, 0)
                if rc != 0:
                    raise RuntimeError(f"axon_start_nrt_profile rc={rc}")
                try:
                    yield
                finally:
                    n = lib.axon_stop_nrt_profile(str(output_dir).encode())
                    print(f"ntff profile: {n} file(s) -> {output_dir}")

            holder[0] = _hook
    except OSError:
        pass
    sys.modules["antenv.axon_hooks"] = mod
    try:
        import antenv

        antenv.axon_hooks = mod
    except ImportError:
        pass


def build_attention_kernel():
    nc = bacc.Bacc(
        "TRN2", target_bir_lowering=False, debug=False, num_devices=NCORES
    )

    xqT = nc.declare_dram_parameter("xqT", [D, QL], BF16, isOutput=False)
    xkT = nc.declare_dram_parameter("xkT", [D, KSH], BF16, isOutput=False)
    xvT = nc.declare_dram_parameter("xvT", [D, KSH], BF16, isOutput=False)
    mskT = nc.declare_dram_parameter("mskT", [KSH, QL], BF16, isOutput=False)
    wqT = nc.declare_dram_parameter("wqT", [D, 2 * HD], BF16, isOutput=False)
    wkT = nc.declare_dram_parameter("wkT", [D, D], BF16, isOutput=False)
    wvT = nc.declare_dram_parameter("wvT", [D, D], BF16, isOutput=False)
    wfT = nc.declare_dram_parameter("wfT", [2 * HD, D], BF16, isOutput=False)
    out = nc.declare_dram_parameter("out", [QL, D], F32, isOutput=True)

    with tile.TileContext(nc) as tc, ExitStack() as ctx:
        consts = ctx.enter_context(tc.tile_pool(name="consts", bufs=1))
        ident = consts.tile([P, P], BF16)
        make_identity(nc, ident)

        # Persistent operand tiles (single-buffered, live for the kernel).
        persist = ctx.enter_context(tc.tile_pool(name="persist", bufs=1))
        wq_sb = persist.tile([P, DB, 2 * HD], BF16)  # [din%, din//, head-col]
        wk_sb = persist.tile([P, DB, D], BF16)
        wv_sb = persist.tile([P, DB, D], BF16)
        wf_sb = persist.tile([P, 2, D], BF16)  # [hd, own-head, dout]
        mask_sb = persist.tile([P, KC, QL], BF16)  # [k, kc, q]
        kT = persist.tile([P, H, KSH], BF16)  # [hd, head, krow]
        v_sb = persist.tile([P, KC, H, HD + 1], BF16)  # [krow, kc, h, hd+1]
        qT = persist.tile([P, H, QL], BF16)  # [hd, qslot, q]
        qmine = persist.tile([P, 2, QL], BF16)  # this core's 2 Q heads
        rsn = [persist.tile([P, QB, HD + 1], BF16, name=f"rsn{i}") for i in range(2)]
        rden = persist.tile([P, 2, QB], F32)
        sumT = persist.tile([P, 2, QL], BF16)  # [hd, own-head, q]

        loads = ctx.enter_context(tc.tile_pool(name="loads", bufs=3))
        probs_pool = ctx.enter_context(tc.tile_pool(name="probs", bufs=3))
        nums = ctx.enter_context(tc.tile_pool(name="nums", bufs=2))
        small = ctx.enter_context(tc.tile_pool(name="small", bufs=4))
        outp = ctx.enter_context(tc.tile_pool(name="outp", bufs=2))
        dram = ctx.enter_context(tc.tile_pool(name="dram", bufs=1, space="DRAM"))

        qag_in = dram.tile([2, P, QL], BF16, name="qag_in")
        qag_out = dram.tile([H, P, QL], BF16, name="qag_out")
        rs_in = [
            dram.tile([4, QB, P, HD + 1], BF16, name=f"rs_in{i}") for i in range(2)
        ]
        rs_out = [
            dram.tile([QB, P, HD + 1], BF16, name=f"rs_out{i}") for i in range(2)
        ]

        # One PSUM pool, 8 banks: mm 2x2 + av 4x1.
        psum = ctx.enter_context(tc.tile_pool(name="psum", bufs=1, space="PSUM"))

        def mm_tile(name, dtype=F32):
            return psum.tile([P, 2, 512], dtype, tag="mm", bufs=2, name=name)

        # --- DMA loads: Q path first, then K so its stream is never starved.
        nc.gpsimd.dma_start(
            out=wq_sb, in_=wqT.rearrange("(a p) m -> p a m", p=P)
        )
        xq_sb = loads.tile([P, DB, QL], BF16, tag="ld", name="xq_sb")
        nc.gpsimd.dma_start(
            out=xq_sb, in_=xqT.rearrange("(a p) q -> p a q", p=P)
        )
        nc.gpsimd.dma_start(
            out=wk_sb, in_=wkT.rearrange("(a p) d -> p a d", p=P)
        )

        # --- Q projection for this core's 2 heads, then AllGather.
        pq = mm_tile("pq")
        for i in range(2):
            for a in range(DB):
                nc.tensor.matmul(
                    pq[:, i, :],
                    wq_sb[:, a, i * HD : (i + 1) * HD],
                    xq_sb[:, a, :],
                    start=(a == 0),
                    stop=(a == DB - 1),
                )
        nc.any.tensor_copy(out=qmine[:], in_=pq[:])
        nc.sync.dma_start(out=qag_in.rearrange("i p q -> p i q"), in_=qmine[:])

        # --- K projection: stream xkT in 512-column chunks.
        def k_chunk_load(c):
            xkc = loads.tile([P, DB, 512], BF16, tag="ld", name=f"xkc{c}")
            nc.gpsimd.dma_start(
                out=xkc,
                in_=xkT[:, c * 512 : (c + 1) * 512].rearrange(
                    "(a p) k -> p a k", p=P
                ),
            )
            return xkc

        xkc0 = k_chunk_load(0)
        xkc1 = k_chunk_load(1)
        nc.gpsimd.collective_compute(
            "AllGather",
            mybir.AluOpType.bypass,
            replica_groups=GROUPS,
            ins=[qag_in.opt()],
            outs=[qag_out.opt()],
        )
        nc.sync.dma_start(out=qT, in_=qag_out.rearrange("s p q -> p s q"))
        xkcs = [xkc0, xkc1, k_chunk_load(2), k_chunk_load(3)]

        for c in range(4):
            xkc = xkcs[c]
            for hp in range(4):
                pk = mm_tile(f"pk_{c}_{hp}")
                for i in range(2):
                    for a in range(DB):
                        nc.tensor.matmul(
                            pk[:, i, :],
                            wk_sb[:, a, hp * 256 + i * HD : hp * 256 + (i + 1) * HD],
                            xkc[:, a, :],
                            start=(a == 0),
                            stop=(a == DB - 1),
                        )
                nc.any.tensor_copy(
                    out=kT[:, 2 * hp : 2 * hp + 2, c * 512 : (c + 1) * 512],
                    in_=pk[:],
                )

        # --- remaining loads (after K chunks on the same DMA queue).
        nc.gpsimd.dma_start(
            out=mask_sb, in_=mskT.rearrange("(a p) q -> p a q", p=P)
        )
        nc.gpsimd.dma_start(
            out=wv_sb, in_=wvT.rearrange("(a p) d -> p a d", p=P)
        )

        # --- V projection: stream xvT in 512-column chunks.
        for c in range(4):
            xvc = loads.tile([P, DB, 512], BF16, tag="ld", name=f"xvc{c}")
            nc.gpsimd.dma_start(
                out=xvc,
                in_=xvT[:, c * 512 : (c + 1) * 512].rearrange(
                    "(a p) k -> p a k", p=P
                ),
            )
            for mkl in range(4):
                mk = c * 4 + mkl
                pv = mm_tile(f"pv_{mk}")
                for n in range(2):
                    for a in range(DB):
                        nc.tensor.matmul(
                            pv[:, n, :],
                            xvc[:, a, mkl * P : (mkl + 1) * P],
                            wv_sb[:, a, n * 512 : (n + 1) * 512],
                            start=(a == 0),
                            stop=(a == DB - 1),
                        )
                nc.any.tensor_copy(
                    out=v_sb[:, mk, :, 0:HD],
                    in_=pv[:].rearrange("p a (b c) -> p (a b) c", b=4),
                )
        nc.vector.memset(v_sb[:, :, :, HD], 1.0)

        nc.gpsimd.dma_start(
            out=wf_sb, in_=wfT.rearrange("(i p) d -> p i d", p=P)
        )

        # --- attention per head; exp batched over 2 k-chunks.
        def attention_head(h):
            avs = [
                psum.tile([P, HD + 1], F32, tag="av", bufs=4, name=f"av_{h}_{qb}")
                for qb in range(QB)
            ]
            for kc2 in range(KC // 2):
                ps = mm_tile(f"ps_{h}_{kc2}")
                for half in range(2):
                    kc = kc2 * 2 + half
                    nc.tensor.matmul(
                        ps[:, half, :],
                        kT[:, h, kc * P : (kc + 1) * P],
                        qT[:, qslot(h), :],
                        start=True,
                        stop=True,
                    )
                probs = probs_pool.tile(
                    [P, 2, 512], BF16, tag="probs", name=f"pr_{h}_{kc2}"
                )
                nc.scalar.activation(
                    probs[:], ps[:], mybir.ActivationFunctionType.Exp, scale=SCALE
                )
                nc.vector.tensor_mul(
                    probs[:], probs[:], mask_sb[:, kc2 * 2 : kc2 * 2 + 2, :]
                )
                for half in range(2):
                    kc = kc2 * 2 + half
                    for qb in range(QB):
                        nc.tensor.matmul(
                            avs[qb][:],
                            probs[:, half, qb * P : (qb + 1) * P],
                            v_sb[:, kc, h, :],
                            start=(kc == 0),
                            stop=(kc == KC - 1),
                        )
            num = nums.tile([P, QB, HD + 1], BF16, tag="num", name=f"num_{h}")
            for qb in range(QB):
                nc.any.tensor_copy(out=num[:, qb, :], in_=avs[qb][:])
            nc.sync.dma_start(
                out=rs_in[h // 4][h % 4].rearrange("b p c -> p b c"), in_=num[:]
            )

        def rs_fire(i):
            nc.gpsimd.collective_compute(
                "ReduceScatter",
                mybir.AluOpType.add,
                replica_groups=GROUPS,
                ins=[rs_in[i].opt()],
                outs=[rs_out[i].opt()],
            )
            nc.gpsimd.dma_start(
                out=rsn[i][:], in_=rs_out[i].rearrange("b p c -> p b c")
            )

        for h in range(4):
            attention_head(h)
        rs_fire(0)
        for h in range(4, H):
            attention_head(h)
        rs_fire(1)

        # --- tail: normalize own 2 heads, transpose, project, store.
        def norm_head(i):
            nc.vector.tensor_copy(out=rden[:, i, :], in_=rsn[i][:, :, HD])
            # guard fully-masked rows (reference wipes them to 0): 0/eps -> 0
            nc.vector.tensor_scalar_max(rden[:, i, :], rden[:, i, :], 1e-30)
            nc.vector.reciprocal(rden[:, i, :], rden[:, i, :])
            snorms = []
            for qb in range(QB):
                snorm = small.tile([P, HD], BF16, tag="snorm", name=f"sn_{i}_{qb}")
                nc.vector.tensor_scalar_mul(
                    snorm[:],
                    rsn[i][:, qb, 0:HD],
                    rden[:, i, qb : qb + 1],
                )
                snorms.append(snorm)
            pst = mm_tile(f"st_{i}", BF16)
            for qb in range(QB):
                nc.tensor.transpose(
                    pst[:, 0, qb * P : (qb + 1) * P], snorms[qb][:], ident
                )
            nc.any.tensor_copy(out=sumT[:, i, :], in_=pst[:, 0, :])

        norm_head(0)

        # Output accumulators: qb 0 on mm (allocated between the two norm
        # transposes so rotation stays deadlock-free), qb 2/3 on the freed
        # AV banks; their i=0 matmuls overlap the second ReduceScatter.
        po0 = mm_tile("po_q0")  # [:, n, :] for qb 0
        po23 = {
            (qb, n): psum.tile(
                [P, 512], F32, tag="av", bufs=4, name=f"po_{qb}_{n}"
            )
            for qb in (2, 3)
            for n in range(2)
        }

        def po_mm(ap, qb, n, i, stop):
            nc.tensor.matmul(
                ap,
                sumT[:, i, qb * P : (qb + 1) * P],
                wf_sb[:, i, n * 512 : (n + 1) * 512],
                start=(i == 0),
                stop=stop,
            )

        for n in range(2):
            po_mm(po0[:, n, :], 0, n, 0, False)
        for (qb, n), po in po23.items():
            po_mm(po[:], qb, n, 0, False)

        norm_head(1)  # stalls on the second ReduceScatter

        engs = [nc.sync, nc.scalar]

        def out_store(qb, ot):
            engs[qb % 2].dma_start(
                out=out[qb * P : (qb + 1) * P, :],
                in_=ot[:].rearrange("p a b -> p (a b)"),
            )

        for n in range(2):
            po_mm(po0[:, n, :], 0, n, 1, True)
        ot0 = outp.tile([P, 2, 512], F32, tag="out", name="ot0")
        nc.any.tensor_copy(out=ot0[:], in_=po0[:])
        out_store(0, ot0)

        for (qb, n), po in po23.items():
            po_mm(po[:], qb, n, 1, True)
        ots = {}
        for qb in (2, 3):
            ots[qb] = outp.tile([P, 2, 512], F32, tag="out", name=f"ot{qb}")
            for n in range(2):
                nc.any.tensor_copy(out=ots[qb][:, n, :], in_=po23[(qb, n)][:])

        po1 = mm_tile("po_q1")  # qb 1 runs last on the freed mm slot
        for i in range(2):
            for n in range(2):
                po_mm(po1[:, n, :], 1, n, i, i == 1)
        ot1 = outp.tile([P, 2, 512], F32, tag="out", name="ot1")
        nc.any.tensor_copy(out=ot1[:], in_=po1[:])
        out_store(1, ot1)
        for qb in (2, 3):
            out_store(qb, ots[qb])

    nc.compile()
    return nc


_NC_CACHE = None


def _get_nc():
    global _NC_CACHE
    if _NC_CACHE is None:
        _NC_CACHE = build_attention_kernel()
    return _NC_CACHE


def make_in_maps(inputs):
    BF = ml_dtypes.bfloat16
    inputs = {k: np.asarray(v) for k, v in inputs.items()}
    WqT = np.asarray(inputs["Wq"]).T.astype(BF)  # [din, dout]
    WkT = np.ascontiguousarray(np.asarray(inputs["Wk"]).T.astype(BF))
    WvT = np.ascontiguousarray(np.asarray(inputs["Wv"]).T.astype(BF))
    WfT = np.asarray(inputs["Wf"]).T.astype(BF)  # [din, dout]
    xqTs = [
        np.ascontiguousarray(inputs["inputs_q"][b].T.astype(BF)) for b in range(B)
    ]
    in_maps = []
    for c in range(NCORES):
        b, g = c // 4, c % 4
        sl = slice(g * KSH, (g + 1) * KSH)
        in_maps.append(
            {
                "xqT": xqTs[b],
                "xkT": np.ascontiguousarray(inputs["inputs_k"][b, sl].T.astype(BF)),
                "xvT": np.ascontiguousarray(inputs["inputs_v"][b, sl].T.astype(BF)),
                "mskT": np.ascontiguousarray(
                    inputs["attention_mask"][b, :, sl].T.astype(BF)
                ),
                "wqT": np.ascontiguousarray(
                    np.concatenate(
                        [
                            WqT[:, g * HD : (g + 1) * HD],
                            WqT[:, (g + 4) * HD : (g + 5) * HD],
                        ],
                        axis=1,
                    )
                ),
                "wkT": WkT,
                "wvT": WvT,
                "wfT": np.ascontiguousarray(
                    np.concatenate(
                        [
                            WfT[g * HD : (g + 1) * HD],
                            WfT[(g + 4) * HD : (g + 5) * HD],
                        ],
                        axis=0,
                    )
                ),
            }
        )
    return in_maps


def gather_out(results):
    out = np.zeros((B, QL, D), np.float32)
    for c in range(NCORES):
        out[c // 4] += results[c]["out"]
    return out


def kernel(**inputs) -> np.ndarray:
    ensure_ntff_hook()  # defensive: BASS_TRACE=1 in env would need the shim
    from concourse.bass_utils import run_bass_kernel_spmd

    nc = _get_nc()
    in_maps = make_in_maps(inputs)
    res = run_bass_kernel_spmd(nc, in_maps, list(range(NCORES)))
    return gather_out(res.results)
